# revision 1
# baseline (speedup 1.0000x reference)
"""ConvGRU (nn_ConvRNN) Trainium2 Bass kernel — 8-core SPMD.

Sharding: 8 cores = (batch n in 0..3) x (H half in {top, bottom}). Each core
owns a 32-row band of one image for the whole pipeline:
  Phase 1: 5x5 conv 64->384 for all T=8 timesteps on the local band (input
           halo rows come pre-sliced from the host), raw conv output spilled
           to DRAM, per-channel sum/sumsq partials reduced on-chip, one
           8-core AllGather for the global BN statistics.
  Phase 2: the GRU-style recurrence. Per step: a 2-row halo AllGather between
           H-half pairs, 3x3 convs (zr: 256ch, hh: 128ch) with one extra
           conv output row computed on each side (so the r*h halo row is
           produced locally), per-conv BN-stat AllGather, fused BN+ReLU6 via
           scalar-engine affine activations.

Conv biases are dropped: BN subtracts the batch mean, so a per-channel bias
added before BN cancels exactly.

Per-core data masks (0/1) from the host make the SPMD program uniform: image
boundary rows multiply to zero, interior halo rows pass through.
"""

import numpy as np

T, N, CIN, H, W = 8, 4, 64, 64, 64
CD = 128
NCORES = 8
EPS = 1e-5

HP1, WP1 = 38, 68        # phase-1 padded input rows/cols per core
ROWS = 34                # conv output rows stored per core (rel 1..34)
OWN = 32                 # own rows
HPAD, WPAD = 36, 66      # phase-2 padded h tile
CNT1 = float(T * N * H * W)      # 131072, x2h BN count
CNT2 = float(N * H * W)          # 16384, recurrence BN count

_PROG = None


def _build_program():
    import concourse.bacc as bacc
    import concourse.bass as bass
    import concourse.tile as tile
    from concourse import mybir

    f32 = mybir.dt.float32
    bf = mybir.dt.float32
    AF = mybir.ActivationFunctionType
    ALU = mybir.AluOpType
    AX = mybir.AxisListType
    PAIRS = [[2 * i, 2 * i + 1] for i in range(NCORES // 2)]
    ALL = [list(range(NCORES))]

    nc = bacc.Bacc("TRN2", target_bir_lowering=False, debug=False,
                   enable_asserts=False, num_devices=NCORES)

    x2d = nc.dram_tensor("x2", [T, 128, HP1, WP1], bf, kind="ExternalInput")
    wpd = nc.dram_tensor("wp", [128, 30, 128], bf, kind="ExternalInput")
    wsd = nc.dram_tensor("ws", [64, 15, 128], bf, kind="ExternalInput")
    wzrd = nc.dram_tensor("wzr", [128, 18, 128], bf, kind="ExternalInput")
    whhd = nc.dram_tensor("whh", [128, 9, 128], bf, kind="ExternalInput")
    gxd = nc.dram_tensor("gx", [128, 3], f32, kind="ExternalInput")
    btxd = nc.dram_tensor("btx", [128, 3], f32, kind="ExternalInput")
    gzrd = nc.dram_tensor("gzr", [128, 2], f32, kind="ExternalInput")
    btzrd = nc.dram_tensor("btzr", [128, 2], f32, kind="ExternalInput")
    ghhd = nc.dram_tensor("ghh", [128, 1], f32, kind="ExternalInput")
    bthhd = nc.dram_tensor("bthh", [128, 1], f32, kind="ExternalInput")
    mpred = nc.dram_tensor("mpre", [128, 1], f32, kind="ExternalInput")
    mpostd = nc.dram_tensor("mpost", [128, 1], f32, kind="ExternalInput")
    outd = nc.dram_tensor("out", [T, 128, OWN, 64], f32, kind="ExternalOutput")

    def ag(ins_ap, outs_ap, groups):
        nc.gpsimd.collective_compute(
            "AllGather", ALU.bypass, replica_groups=groups,
            ins=[ins_ap], outs=[outs_ap])

    with tile.TileContext(nc) as tc:
        with tc.tile_pool(name="consts", bufs=1) as consts, \
             tc.tile_pool(name="dram", bufs=1, space="DRAM") as dram, \
             tc.tile_pool(name="dram2", bufs=2, space="DRAM") as dram2, \
             tc.tile_pool(name="stp", bufs=2) as stp:

            # ---- persistent weights / consts ----
            wp_sb = consts.tile([128, 30, 128], bf)
            nc.sync.dma_start(wp_sb, wpd[:])
            ws_sb = consts.tile([64, 15, 128], bf)
            nc.sync.dma_start(ws_sb, wsd[:])
            wzr_sb = consts.tile([128, 18, 128], bf)
            nc.sync.dma_start(wzr_sb, wzrd[:])
            whh_sb = consts.tile([128, 9, 128], bf)
            nc.sync.dma_start(whh_sb, whhd[:])
            gx_sb = consts.tile([128, 3], f32)
            nc.sync.dma_start(gx_sb, gxd[:])
            btx_sb = consts.tile([128, 3], f32)
            nc.sync.dma_start(btx_sb, btxd[:])
            gzr_sb = consts.tile([128, 2], f32)
            nc.sync.dma_start(gzr_sb, gzrd[:])
            btzr_sb = consts.tile([128, 2], f32)
            nc.sync.dma_start(btzr_sb, btzrd[:])
            ghh_sb = consts.tile([128, 1], f32)
            nc.sync.dma_start(ghh_sb, ghhd[:])
            bthh_sb = consts.tile([128, 1], f32)
            nc.sync.dma_start(bthh_sb, bthhd[:])
            mpre_sb = consts.tile([128, 1], f32)
            nc.sync.dma_start(mpre_sb, mpred[:])
            mpost_sb = consts.tile([128, 1], f32)
            nc.sync.dma_start(mpost_sb, mpostd[:])

            eps_sb = consts.tile([128, 1], f32)
            nc.vector.memset(eps_sb, EPS)
            h_pad = consts.tile([128, HPAD, WPAD], f32)
            nc.vector.memset(h_pad, 0.0)
            rh_pad = consts.tile([128, HPAD, WPAD], bf)
            nc.vector.memset(rh_pad, 0.0)
            h_bf = consts.tile([128, HPAD, WPAD], bf)
            nc.vector.memset(h_bf, 0.0)

            y_dram = dram.tile([T, 3, 128, ROWS, 64], f32)

            stats1 = consts.tile([128, 3, T, 8], f32)
            scale1 = consts.tile([128, 3], f32)
            shift1 = consts.tile([128, 3], f32)

            # ================= Phase 1: 5x5 conv 64->384 =================
            with tc.tile_pool(name="x1", bufs=2) as x1, \
                 tc.tile_pool(name="y1", bufs=3) as y1, \
                 tc.tile_pool(name="sq1", bufs=2) as sq1, \
                 tc.tile_pool(name="ps1", bufs=6, space="PSUM") as ps1, \
                 tc.tile_pool(name="pse1", bufs=2, space="PSUM") as pse1:
                for t in range(T):
                    x2t = x1.tile([128, HP1, WP1], bf, tag="x2t")
                    for r0, r1 in ((0, 10), (10, 20), (20, 30), (30, 38)):
                        nc.sync.dma_start(x2t[:, r0:r1, :], x2d[t, :, r0:r1, :])
                    for c3 in range(3):
                        y_sb = y1.tile([128, ROWS, 64], f32, tag="y1t")
                        for ft in range(4):
                            i0 = 1 + 8 * ft
                            ps = ps1.tile([128, 8, 64], f32, tag="ps1")
                            k = 0
                            for ky in range(5):
                                for e in range(2):
                                    nc.tensor.matmul(
                                        ps, wp_sb[:, c3 * 10 + ky * 2 + e, :],
                                        x2t[:, i0 + ky:i0 + ky + 8, 2 * e:2 * e + 64],
                                        start=(k == 0), stop=False)
                                    k += 1
                                nc.tensor.matmul(
                                    ps, ws_sb[:, c3 * 5 + ky, :],
                                    x2t[0:64, i0 + ky:i0 + ky + 8, 4:68],
                                    start=False, stop=(ky == 4))
                                k += 1
                            # evacuate raw + per-channel sums
                            nc.vector.tensor_scalar(
                                out=y_sb[:, i0:i0 + 8, :], in0=ps,
                                scalar1=1.0, scalar2=None, op0=ALU.mult, op1=ALU.add,
                                accum_out=stats1[:, c3, t, ft:ft + 1])
                            sq = sq1.tile([128, 8, 64], f32, tag="sq")
                            nc.scalar.activation(
                                sq, ps, AF.Square,
                                accum_out=stats1[:, c3, t, 4 + ft:5 + ft])
                        # edge rows {0, 33} (rel 1 and 34)
                        pse = pse1.tile([128, 2, 64], f32, tag="pse")
                        k = 0
                        for ky in range(5):
                            for e in range(2):
                                nc.tensor.matmul(
                                    pse, wp_sb[:, c3 * 10 + ky * 2 + e, :],
                                    x2t[:, ky::33, 2 * e:2 * e + 64],
                                    start=(k == 0), stop=False)
                                k += 1
                            nc.tensor.matmul(
                                pse, ws_sb[:, c3 * 5 + ky, :],
                                x2t[0:64, ky::33, 4:68],
                                start=False, stop=(ky == 4))
                            k += 1
                        nc.vector.tensor_copy(y_sb[:, 0:1, :], pse[:, 0:1, :])
                        nc.vector.tensor_copy(y_sb[:, 33:34, :], pse[:, 1:2, :])
                        nc.sync.dma_start(y_dram[t, c3, :, 0:17, :], y_sb[:, 0:17, :])
                        nc.sync.dma_start(y_dram[t, c3, :, 17:34, :], y_sb[:, 17:34, :])

            # ---- phase-1 BN stats: local reduce -> AllGather -> combine ----
            stin1 = stp.tile([128, 3, 2], f32, tag="stin1")
            for c3 in range(3):
                nc.vector.reduce_sum(stin1[:, c3, 0:1], stats1[:, c3, :, 0:4], axis=AX.XY)
                nc.vector.reduce_sum(stin1[:, c3, 1:2], stats1[:, c3, :, 4:8], axis=AX.XY)
            ag1i = dram.tile([128, 3, 2], f32)
            ag1o = dram.tile([NCORES, 128, 6], f32)
            nc.sync.dma_start(ag1i, stin1)
            ag(ag1i.opt(), ag1o.opt(), ALL)
            g1 = stp.tile([128, 6, NCORES], f32, tag="g1")
            import concourse.bass as _b
            nc.sync.dma_start(
                g1, _b.AP(tensor=ag1o.tensor, offset=ag1o.offset,
                          ap=[[6, 128], [1, 6], [768, NCORES]]))
            tot1 = stp.tile([128, 6], f32, tag="tot1")
            nc.vector.reduce_sum(tot1, g1, axis=AX.X)
            mean1 = stp.tile([128, 3], f32, tag="mean1")
            nc.vector.tensor_scalar_mul(mean1, tot1[:, 0::2], 1.0 / CNT1)
            e2 = stp.tile([128, 3], f32, tag="e2")
            nc.vector.tensor_scalar_mul(e2, tot1[:, 1::2], 1.0 / CNT1)
            m2 = stp.tile([128, 3], f32, tag="m2")
            nc.vector.tensor_mul(m2, mean1, mean1)
            nc.vector.tensor_sub(e2, e2, m2)
            nc.scalar.activation(e2, e2, AF.Sqrt, bias=eps_sb)
            nc.vector.reciprocal(e2, e2)
            nc.vector.tensor_mul(scale1, gx_sb, e2)
            nc.vector.tensor_mul(m2, mean1, scale1)
            nc.vector.tensor_sub(shift1, btx_sb, m2)

            # ================= Phase 2: recurrence =================
            with tc.tile_pool(name="yp", bufs=4) as yp, \
                 tc.tile_pool(name="crawp", bufs=4) as crawp, \
                 tc.tile_pool(name="tp", bufs=2) as tp, \
                 tc.tile_pool(name="sq2", bufs=2) as sq2, \
                 tc.tile_pool(name="ps2", bufs=6, space="PSUM") as ps2, \
                 tc.tile_pool(name="pse2", bufs=2, space="PSUM") as pse2:

                # ---- t = 0: h0 = sigmoid(x_z) * tanh(x_h) ----
                y0z = yp.tile([128, ROWS, 64], f32, tag="yt")
                nc.sync.dma_start(y0z[:, 0:17, :], y_dram[0, 0, :, 0:17, :])
                nc.sync.dma_start(y0z[:, 17:34, :], y_dram[0, 0, :, 17:34, :])
                y0h = yp.tile([128, ROWS, 64], f32, tag="yt")
                nc.sync.dma_start(y0h[:, 0:17, :], y_dram[0, 2, :, 0:17, :])
                nc.sync.dma_start(y0h[:, 17:34, :], y_dram[0, 2, :, 17:34, :])
                z0 = tp.tile([128, OWN, 64], f32, tag="z")
                nc.scalar.activation(z0, y0z[:, 1:33, :], AF.Sigmoid,
                                     scale=scale1[:, 0:1], bias=shift1[:, 0:1])
                th0 = tp.tile([128, OWN, 64], f32, tag="d")
                nc.scalar.activation(th0, y0h[:, 1:33, :], AF.Tanh,
                                     scale=scale1[:, 2:3], bias=shift1[:, 2:3])
                nc.vector.tensor_mul(h_pad[:, 2:34, 1:65], z0, th0)
                nc.vector.tensor_copy(h_bf[:, 2:34, 1:65], h_pad[:, 2:34, 1:65])
                nc.sync.dma_start(outd[0], h_pad[:, 2:34, 1:65])

                for t in range(1, T):
                    # ---- halo AllGather of h edge rows between H-half pairs ----
                    hbi = dram2.tile([128, 4, 64], f32, tag="hbi")
                    nc.sync.dma_start(hbi[:, 0:2, :], h_pad[:, 2:4, 1:65])
                    nc.sync.dma_start(hbi[:, 2:4, :], h_pad[:, 32:34, 1:65])
                    hbo = dram2.tile([2, 128, 4, 64], f32, tag="hbo")
                    ag(hbi.opt(), hbo.opt(), PAIRS)
                    nc.sync.dma_start(h_pad[:, 0:2, 1:65], hbo[0, :, 2:4, :])
                    nc.vector.tensor_scalar_mul(h_pad[:, 0:2, 1:65],
                                                h_pad[:, 0:2, 1:65], mpre_sb)
                    nc.vector.tensor_copy(h_bf[:, 0:2, 1:65], h_pad[:, 0:2, 1:65])
                    nc.sync.dma_start(h_pad[:, 34:36, 1:65], hbo[1, :, 0:2, :])
                    nc.vector.tensor_scalar_mul(h_pad[:, 34:36, 1:65],
                                                h_pad[:, 34:36, 1:65], mpost_sb)
                    nc.vector.tensor_copy(h_bf[:, 34:36, 1:65], h_pad[:, 34:36, 1:65])

                    # ---- y loads ----
                    yz = yp.tile([128, ROWS, 64], f32, tag="yt")
                    nc.sync.dma_start(yz[:, 0:17, :], y_dram[t, 0, :, 0:17, :])
                    nc.sync.dma_start(yz[:, 17:34, :], y_dram[t, 0, :, 17:34, :])
                    yr = yp.tile([128, ROWS, 64], f32, tag="yt")
                    nc.sync.dma_start(yr[:, 0:17, :], y_dram[t, 1, :, 0:17, :])
                    nc.sync.dma_start(yr[:, 17:34, :], y_dram[t, 1, :, 17:34, :])
                    yh = yp.tile([128, ROWS, 64], f32, tag="yt")
                    nc.sync.dma_start(yh[:, 0:17, :], y_dram[t, 2, :, 0:17, :])
                    nc.sync.dma_start(yh[:, 17:34, :], y_dram[t, 2, :, 17:34, :])

                    # ---- conv_zr: 3x3, h -> 256ch, rows rel 1..34 ----
                    st_zr = stp.tile([128, 2, 8], f32, tag="stzr")
                    craw = []
                    for c2 in range(2):
                        cr = crawp.tile([128, ROWS, 64], f32, tag="craw")
                        craw.append(cr)
                        for ft in range(4):
                            i0 = 1 + 8 * ft
                            ps = ps2.tile([128, 8, 64], f32, tag="ps2")
                            k = 0
                            for ky in range(3):
                                for kx in range(3):
                                    nc.tensor.matmul(
                                        ps, wzr_sb[:, c2 * 9 + ky * 3 + kx, :],
                                        h_bf[:, i0 + ky:i0 + ky + 8, kx:kx + 64],
                                        start=(k == 0), stop=(k == 8))
                                    k += 1
                            nc.vector.tensor_scalar(
                                out=cr[:, i0:i0 + 8, :], in0=ps,
                                scalar1=1.0, scalar2=None, op0=ALU.mult, op1=ALU.add,
                                accum_out=st_zr[:, c2, ft:ft + 1])
                            sq = sq2.tile([128, 8, 64], f32, tag="sq2")
                            nc.scalar.activation(
                                sq, ps, AF.Square,
                                accum_out=st_zr[:, c2, 4 + ft:5 + ft])
                        pse = pse2.tile([128, 2, 64], f32, tag="pse2")
                        k = 0
                        for ky in range(3):
                            for kx in range(3):
                                nc.tensor.matmul(
                                    pse, wzr_sb[:, c2 * 9 + ky * 3 + kx, :],
                                    h_bf[:, ky::33, kx:kx + 64],
                                    start=(k == 0), stop=(k == 8))
                                k += 1
                        nc.vector.tensor_copy(craw[c2][:, 0:1, :], pse[:, 0:1, :])
                        nc.vector.tensor_copy(craw[c2][:, 33:34, :], pse[:, 1:2, :])

                    # ---- zr BN stats AllGather ----
                    stv = stp.tile([128, 2, 2], f32, tag="stv")
                    for c2 in range(2):
                        nc.vector.reduce_sum(stv[:, c2, 0:1], st_zr[:, c2, 0:4], axis=AX.X)
                        nc.vector.reduce_sum(stv[:, c2, 1:2], st_zr[:, c2, 4:8], axis=AX.X)
                    sti = dram2.tile([128, 2, 2], f32, tag="sti")
                    nc.sync.dma_start(sti, stv)
                    sto = dram2.tile([NCORES, 128, 4], f32, tag="sto")
                    ag(sti.opt(), sto.opt(), ALL)
                    g2 = stp.tile([128, 4, NCORES], f32, tag="g2")
                    nc.sync.dma_start(
                        g2, _b.AP(tensor=sto.tensor, offset=sto.offset,
                                  ap=[[4, 128], [1, 4], [512, NCORES]]))
                    tot2 = stp.tile([128, 4], f32, tag="tot2")
                    nc.vector.reduce_sum(tot2, g2, axis=AX.X)
                    mean2 = stp.tile([128, 2], f32, tag="mean2")
                    nc.vector.tensor_scalar_mul(mean2, tot2[:, 0::2], 1.0 / CNT2)
                    e22 = stp.tile([128, 2], f32, tag="e22")
                    nc.vector.tensor_scalar_mul(e22, tot2[:, 1::2], 1.0 / CNT2)
                    m22 = stp.tile([128, 2], f32, tag="m22")
                    nc.vector.tensor_mul(m22, mean2, mean2)
                    nc.vector.tensor_sub(e22, e22, m22)
                    nc.scalar.activation(e22, e22, AF.Sqrt, bias=eps_sb)
                    nc.vector.reciprocal(e22, e22)
                    sc2 = stp.tile([128, 2], f32, tag="sc2")
                    nc.vector.tensor_mul(sc2, gzr_sb, e22)
                    sh2 = stp.tile([128, 2], f32, tag="sh2")
                    nc.vector.tensor_mul(m22, mean2, sc2)
                    nc.vector.tensor_sub(sh2, btzr_sb, m22)

                    # ---- tmp = relu6(BN(conv)) + BN_x2h(y); z, r; rh ----
                    for c2 in range(2):
                        cr = craw[c2]
                        nc.scalar.activation(cr, cr, AF.Relu,
                                             scale=sc2[:, c2:c2 + 1],
                                             bias=sh2[:, c2:c2 + 1])
                        nc.vector.tensor_scalar_min(cr, cr, 6.0)
                        yt_ = yz if c2 == 0 else yr
                        nc.vector.tensor_scalar(
                            out=yt_, in0=yt_, scalar1=scale1[:, c2:c2 + 1],
                            scalar2=shift1[:, c2:c2 + 1],
                            op0=ALU.mult, op1=ALU.add)
                        nc.vector.tensor_add(cr, cr, yt_)
                    z = tp.tile([128, OWN, 64], f32, tag="z")
                    nc.scalar.activation(z, craw[0][:, 1:33, :], AF.Sigmoid)
                    r = craw[1]
                    nc.scalar.activation(r, r, AF.Sigmoid)
                    nc.vector.tensor_mul(rh_pad[:, 2:34, 1:65], r[:, 1:33, :],
                                         h_pad[:, 2:34, 1:65])
                    nc.vector.tensor_mul(rh_pad[:, 1:2, 1:65], r[:, 0:1, :],
                                         h_pad[:, 1:2, 1:65])
                    nc.vector.tensor_scalar_mul(rh_pad[:, 1:2, 1:65],
                                                rh_pad[:, 1:2, 1:65], mpre_sb)
                    nc.vector.tensor_mul(rh_pad[:, 34:35, 1:65], r[:, 33:34, :],
                                         h_pad[:, 34:35, 1:65])
                    nc.vector.tensor_scalar_mul(rh_pad[:, 34:35, 1:65],
                                                rh_pad[:, 34:35, 1:65], mpost_sb)

                    # ---- conv_hh: 3x3, rh -> 128ch, own rows ----
                    st_hh = stp.tile([128, 8], f32, tag="sthh")
                    ch = crawp.tile([128, ROWS, 64], f32, tag="craw")
                    for ft in range(4):
                        j0 = 8 * ft
                        ps = ps2.tile([128, 8, 64], f32, tag="ps2")
                        k = 0
                        for ky in range(3):
                            for kx in range(3):
                                nc.tensor.matmul(
                                    ps, whh_sb[:, ky * 3 + kx, :],
                                    rh_pad[:, j0 + 1 + ky:j0 + 1 + ky + 8, kx:kx + 64],
                                    start=(k == 0), stop=(k == 8))
                                k += 1
                        nc.vector.tensor_scalar(
                            out=ch[:, j0:j0 + 8, :], in0=ps,
                            scalar1=1.0, scalar2=None, op0=ALU.mult, op1=ALU.add,
                            accum_out=st_hh[:, ft:ft + 1])
                        sq = sq2.tile([128, 8, 64], f32, tag="sq2")
                        nc.scalar.activation(sq, ps, AF.Square,
                                             accum_out=st_hh[:, 4 + ft:5 + ft])

                    # ---- hh BN stats AllGather ----
                    stv2 = stp.tile([128, 2], f32, tag="stv2")
                    nc.vector.reduce_sum(stv2[:, 0:1], st_hh[:, 0:4], axis=AX.X)
                    nc.vector.reduce_sum(stv2[:, 1:2], st_hh[:, 4:8], axis=AX.X)
                    sti2 = dram2.tile([128, 2], f32, tag="sti2")
                    nc.sync.dma_start(sti2, stv2)
                    sto2 = dram2.tile([NCORES, 128, 2], f32, tag="sto2")
                    ag(sti2.opt(), sto2.opt(), ALL)
                    g3 = stp.tile([128, 2, NCORES], f32, tag="g3")
                    nc.sync.dma_start(
                        g3, _b.AP(tensor=sto2.tensor, offset=sto2.offset,
                                  ap=[[2, 128], [1, 2], [256, NCORES]]))
                    tot3 = stp.tile([128, 2], f32, tag="tot3")
                    nc.vector.reduce_sum(tot3, g3, axis=AX.X)
                    mean3 = stp.tile([128, 1], f32, tag="mean3")
                    nc.vector.tensor_scalar_mul(mean3, tot3[:, 0:1], 1.0 / CNT2)
                    e23 = stp.tile([128, 1], f32, tag="e23")
                    nc.vector.tensor_scalar_mul(e23, tot3[:, 1:2], 1.0 / CNT2)
                    m23 = stp.tile([128, 1], f32, tag="m23")
                    nc.vector.tensor_mul(m23, mean3, mean3)
                    nc.vector.tensor_sub(e23, e23, m23)
                    nc.scalar.activation(e23, e23, AF.Sqrt, bias=eps_sb)
                    nc.vector.reciprocal(e23, e23)
                    sc3 = stp.tile([128, 1], f32, tag="sc3")
                    nc.vector.tensor_mul(sc3, ghh_sb, e23)
                    sh3 = stp.tile([128, 1], f32, tag="sh3")
                    nc.vector.tensor_mul(m23, mean3, sc3)
                    nc.vector.tensor_sub(sh3, bthh_sb, m23)

                    # ---- hh -> tanh -> nh; update h in place ----
                    nc.scalar.activation(ch[:, 0:32, :], ch[:, 0:32, :], AF.Relu,
                                         scale=sc3, bias=sh3)
                    nc.vector.tensor_scalar_min(ch[:, 0:32, :], ch[:, 0:32, :], 6.0)
                    nc.vector.tensor_scalar(
                        out=yh, in0=yh, scalar1=scale1[:, 2:3],
                        scalar2=shift1[:, 2:3], op0=ALU.mult, op1=ALU.add)
                    nc.vector.tensor_add(ch[:, 0:32, :], ch[:, 0:32, :], yh[:, 1:33, :])
                    nc.scalar.activation(ch[:, 0:32, :], ch[:, 0:32, :], AF.Tanh)
                    d = tp.tile([128, OWN, 64], f32, tag="d")
                    nc.vector.tensor_sub(d, ch[:, 0:32, :], h_pad[:, 2:34, 1:65])
                    nc.vector.tensor_mul(d, z, d)
                    nc.vector.tensor_add(h_pad[:, 2:34, 1:65],
                                         h_pad[:, 2:34, 1:65], d)
                    nc.vector.tensor_copy(h_bf[:, 2:34, 1:65], h_pad[:, 2:34, 1:65])
                    nc.sync.dma_start(outd[t], h_pad[:, 2:34, 1:65])

    nc.compile()
    return nc


def get_prog():
    global _PROG
    if _PROG is None:
        _PROG = _build_program()
    return _PROG


def prep_in_maps(x, w_x2h, g_x2h, bt_x2h, w_zr, g_zr, bt_zr, w_hh, g_hh, bt_hh):
    """Shard + pre-transform inputs on the host. Returns list of per-core dicts."""
    x = np.asarray(x, np.float32)
    w_x2h = np.asarray(w_x2h, np.float32)
    w_zr = np.asarray(w_zr, np.float32)
    w_hh = np.asarray(w_hh, np.float32)

    # weights: pairs / singles for the 5x5, per-tap for the 3x3s
    wp = np.zeros((128, 30, 128), np.float32)
    ws = np.zeros((64, 15, 128), np.float32)
    for c3 in range(3):
        cs = slice(128 * c3, 128 * (c3 + 1))
        for ky in range(5):
            for e in range(2):
                kx = 2 * e
                wp[0:64, c3 * 10 + ky * 2 + e] = w_x2h[cs, :, ky, kx].T
                wp[64:128, c3 * 10 + ky * 2 + e] = w_x2h[cs, :, ky, kx + 1].T
            ws[:, c3 * 5 + ky] = w_x2h[cs, :, ky, 4].T
    wzr = np.zeros((128, 18, 128), np.float32)
    for c2 in range(2):
        cs = slice(128 * c2, 128 * (c2 + 1))
        for ky in range(3):
            for kx in range(3):
                wzr[:, c2 * 9 + ky * 3 + kx] = w_zr[cs, :, ky, kx].T
    whh = np.zeros((128, 9, 128), np.float32)
    for ky in range(3):
        for kx in range(3):
            whh[:, ky * 3 + kx] = w_hh[:, :, ky, kx].T

    gx = np.ascontiguousarray(np.asarray(g_x2h, np.float32).reshape(3, 128).T)
    btx = np.ascontiguousarray(np.asarray(bt_x2h, np.float32).reshape(3, 128).T)
    gzr = np.ascontiguousarray(np.asarray(g_zr, np.float32).reshape(2, 128).T)
    btzr = np.ascontiguousarray(np.asarray(bt_zr, np.float32).reshape(2, 128).T)
    ghh = np.asarray(g_hh, np.float32).reshape(128, 1)
    bthh = np.asarray(bt_hh, np.float32).reshape(128, 1)

    shared = dict(wp=wp, ws=ws, wzr=wzr, whh=whh, gx=gx, btx=btx,
                  gzr=gzr, btzr=btzr, ghh=ghh, bthh=bthh)

    in_maps = []
    for c in range(NCORES):
        n, half = c // 2, c % 2
        base = 32 * half
        # padded input band: rows base-3..base+34, cols -2..65
        xpad = np.zeros((T, CIN, HP1, WP1), np.float32)
        i0 = max(0, base - 3)
        i1 = min(H, base + 35)
        j0 = i0 - (base - 3)
        xpad[:, :, j0:j0 + (i1 - i0), 2:66] = x[:, n, :, i0:i1, :]
        x2 = np.zeros((T, 128, HP1, WP1), np.float32)
        x2[:, 0:64] = xpad
        x2[:, 64:128, :, 0:WP1 - 1] = xpad[:, :, :, 1:WP1]
        m = dict(shared)
        m["x2"] = x2
        m["mpre"] = np.full((128, 1), 0.0 if half == 0 else 1.0, np.float32)
        m["mpost"] = np.full((128, 1), 1.0 if half == 0 else 0.0, np.float32)
        in_maps.append(m)
    return in_maps


def assemble_output(results):
    out = np.zeros((T, N, CD, H, W), np.float32)
    for c in range(NCORES):
        n, half = c // 2, c % 2
        out[:, n, :, 32 * half:32 * half + 32, :] = results[c]["out"]
    return out


def kernel(**inputs):
    from concourse import bass_utils
    nc = get_prog()
    in_maps = prep_in_maps(
        inputs["x"], inputs["w_x2h"], inputs["g_x2h"], inputs["bt_x2h"],
        inputs["w_zr"], inputs["g_zr"], inputs["bt_zr"],
        inputs["w_hh"], inputs["g_hh"], inputs["bt_hh"])
    res = bass_utils.run_bass_kernel_spmd(nc, in_maps, core_ids=list(range(NCORES)))
    return assemble_output(res.results)



# revision 4
# speedup vs baseline: 1.8995x; 1.8995x over previous
"""ConvGRU (nn_ConvRNN) Trainium2 Bass kernel — 8-core SPMD.

Sharding: 8 cores = (batch n in 0..3) x (H half in {top, bottom}). Each core
owns a 32-row band of one image for the whole pipeline:
  Phase 1: 5x5 conv 64->384 for all T=8 timesteps on the local band (input
           halo rows come pre-sliced from the host), raw conv output spilled
           to DRAM, per-channel sum/sumsq partials reduced on-chip, one
           8-core AllGather for the global BN statistics.
  Phase 2: the GRU-style recurrence. Per step: a 2-row halo AllGather between
           H-half pairs, 3x3 convs (zr: 256ch, hh: 128ch) with one extra
           conv output row computed on each side (so the r*h halo row is
           produced locally), per-conv BN-stat AllGather, fused BN+ReLU6 via
           scalar-engine affine activations.

Conv biases are dropped: BN subtracts the batch mean, so a per-channel bias
added before BN cancels exactly.

Per-core data masks (0/1) from the host make the SPMD program uniform: image
boundary rows multiply to zero, interior halo rows pass through.
"""

import numpy as np

T, N, CIN, H, W = 8, 4, 64, 64, 64
CD = 128
NCORES = 8
EPS = 1e-5

HP1, WP1 = 38, 68        # phase-1 padded input rows/cols per core
ROWS = 34                # conv output rows stored per core (rel 1..34)
OWN = 32                 # own rows
HPAD, WPAD = 36, 66      # phase-2 padded h tile
CNT1 = float(T * N * H * W)      # 131072, x2h BN count
CNT2 = float(N * H * W)          # 16384, recurrence BN count

_PROG = None


def _build_program():
    import concourse.bacc as bacc
    import concourse.bass as bass
    import concourse.tile as tile
    from concourse import mybir

    f32 = mybir.dt.float32
    bf = mybir.dt.bfloat16
    AF = mybir.ActivationFunctionType
    ALU = mybir.AluOpType
    AX = mybir.AxisListType
    PAIRS = [[2 * i, 2 * i + 1] for i in range(NCORES // 2)]
    ALL = [list(range(NCORES))]

    nc = bacc.Bacc("TRN2", target_bir_lowering=False, debug=False,
                   enable_asserts=False, num_devices=NCORES)

    x2d = nc.dram_tensor("x2", [T, 128, HP1, WP1], bf, kind="ExternalInput")
    wpd = nc.dram_tensor("wp", [128, 30, 128], bf, kind="ExternalInput")
    wsd = nc.dram_tensor("ws", [64, 15, 128], bf, kind="ExternalInput")
    wzrd = nc.dram_tensor("wzr", [128, 18, 128], bf, kind="ExternalInput")
    whhd = nc.dram_tensor("whh", [128, 9, 128], bf, kind="ExternalInput")
    gxd = nc.dram_tensor("gx", [128, 3], f32, kind="ExternalInput")
    btxd = nc.dram_tensor("btx", [128, 3], f32, kind="ExternalInput")
    gzrd = nc.dram_tensor("gzr", [128, 2], f32, kind="ExternalInput")
    btzrd = nc.dram_tensor("btzr", [128, 2], f32, kind="ExternalInput")
    ghhd = nc.dram_tensor("ghh", [128, 1], f32, kind="ExternalInput")
    bthhd = nc.dram_tensor("bthh", [128, 1], f32, kind="ExternalInput")
    mpred = nc.dram_tensor("mpre", [128, 1], f32, kind="ExternalInput")
    mpostd = nc.dram_tensor("mpost", [128, 1], f32, kind="ExternalInput")
    outd = nc.dram_tensor("out", [T, 128, OWN, 64], f32, kind="ExternalOutput")

    def ag(ins_ap, outs_ap, groups):
        nc.gpsimd.collective_compute(
            "AllGather", ALU.bypass, replica_groups=groups,
            ins=[ins_ap], outs=[outs_ap])

    with tile.TileContext(nc) as tc:
        with tc.tile_pool(name="consts", bufs=1) as consts, \
             tc.tile_pool(name="dram", bufs=1, space="DRAM") as dram, \
             tc.tile_pool(name="dram2", bufs=2, space="DRAM") as dram2, \
             tc.tile_pool(name="stp", bufs=2) as stp:

            # ---- persistent weights / consts ----
            wp_sb = consts.tile([128, 30, 128], bf)
            nc.sync.dma_start(wp_sb, wpd[:])
            ws_sb = consts.tile([64, 15, 128], bf)
            nc.sync.dma_start(ws_sb, wsd[:])
            wzr_sb = consts.tile([128, 18, 128], bf)
            nc.sync.dma_start(wzr_sb, wzrd[:])
            whh_sb = consts.tile([128, 9, 128], bf)
            nc.sync.dma_start(whh_sb, whhd[:])
            gx_sb = consts.tile([128, 3], f32)
            nc.sync.dma_start(gx_sb, gxd[:])
            btx_sb = consts.tile([128, 3], f32)
            nc.sync.dma_start(btx_sb, btxd[:])
            gzr_sb = consts.tile([128, 2], f32)
            nc.sync.dma_start(gzr_sb, gzrd[:])
            btzr_sb = consts.tile([128, 2], f32)
            nc.sync.dma_start(btzr_sb, btzrd[:])
            ghh_sb = consts.tile([128, 1], f32)
            nc.sync.dma_start(ghh_sb, ghhd[:])
            bthh_sb = consts.tile([128, 1], f32)
            nc.sync.dma_start(bthh_sb, bthhd[:])
            mpre_sb = consts.tile([128, 1], f32)
            nc.sync.dma_start(mpre_sb, mpred[:])
            mpost_sb = consts.tile([128, 1], f32)
            nc.sync.dma_start(mpost_sb, mpostd[:])

            eps_sb = consts.tile([128, 1], f32)
            nc.vector.memset(eps_sb, EPS)
            h_pad = consts.tile([128, HPAD, WPAD], f32)
            nc.vector.memset(h_pad, 0.0)
            rh_pad = consts.tile([128, HPAD, WPAD], bf)
            nc.vector.memset(rh_pad, 0.0)
            h_bf = consts.tile([128, HPAD, WPAD], bf)
            nc.vector.memset(h_bf, 0.0)

            y_dram = dram.tile([T, 3, 128, ROWS, 64], f32)

            stats1 = consts.tile([128, 3, T, 8], f32)
            scale1 = consts.tile([128, 3], f32)
            shift1 = consts.tile([128, 3], f32)

            # ================= Phase 1: 5x5 conv 64->384 =================
            with tc.tile_pool(name="x1", bufs=2) as x1, \
                 tc.tile_pool(name="y1", bufs=3) as y1, \
                 tc.tile_pool(name="sq1", bufs=2) as sq1, \
                 tc.tile_pool(name="ps1", bufs=6, space="PSUM") as ps1, \
                 tc.tile_pool(name="pse1", bufs=2, space="PSUM") as pse1:
                for t in range(T):
                    x2t = x1.tile([128, HP1, WP1], bf, tag="x2t")
                    for r0, r1 in ((0, 10), (10, 20), (20, 30), (30, 38)):
                        nc.sync.dma_start(x2t[:, r0:r1, :], x2d[t, :, r0:r1, :])
                    for c3 in range(3):
                        y_sb = y1.tile([128, ROWS, 64], f32, tag="y1t")
                        for ft in range(4):
                            i0 = 1 + 8 * ft
                            ps = ps1.tile([128, 8, 64], f32, tag="ps1")
                            k = 0
                            for ky in range(5):
                                for e in range(2):
                                    nc.tensor.matmul(
                                        ps, wp_sb[:, c3 * 10 + ky * 2 + e, :],
                                        x2t[:, i0 + ky:i0 + ky + 8, 2 * e:2 * e + 64],
                                        start=(k == 0), stop=False)
                                    k += 1
                                nc.tensor.matmul(
                                    ps, ws_sb[:, c3 * 5 + ky, :],
                                    x2t[0:64, i0 + ky:i0 + ky + 8, 4:68],
                                    start=False, stop=(ky == 4))
                                k += 1
                            # evacuate raw + per-channel sums
                            nc.vector.tensor_scalar(
                                out=y_sb[:, i0:i0 + 8, :], in0=ps,
                                scalar1=1.0, scalar2=None, op0=ALU.mult, op1=ALU.add,
                                accum_out=stats1[:, c3, t, ft:ft + 1])
                            sq = sq1.tile([128, 8, 64], f32, tag="sq")
                            nc.scalar.activation(
                                sq, ps, AF.Square,
                                accum_out=stats1[:, c3, t, 4 + ft:5 + ft])
                        # edge rows {0, 33} (rel 1 and 34)
                        pse = pse1.tile([128, 2, 64], f32, tag="pse")
                        k = 0
                        for ky in range(5):
                            for e in range(2):
                                nc.tensor.matmul(
                                    pse, wp_sb[:, c3 * 10 + ky * 2 + e, :],
                                    x2t[:, ky::33, 2 * e:2 * e + 64],
                                    start=(k == 0), stop=False)
                                k += 1
                            nc.tensor.matmul(
                                pse, ws_sb[:, c3 * 5 + ky, :],
                                x2t[0:64, ky::33, 4:68],
                                start=False, stop=(ky == 4))
                            k += 1
                        nc.vector.tensor_copy(y_sb[:, 0:1, :], pse[:, 0:1, :])
                        nc.vector.tensor_copy(y_sb[:, 33:34, :], pse[:, 1:2, :])
                        nc.sync.dma_start(y_dram[t, c3, :, 0:17, :], y_sb[:, 0:17, :])
                        nc.sync.dma_start(y_dram[t, c3, :, 17:34, :], y_sb[:, 17:34, :])

            # ---- phase-1 BN stats: local reduce -> AllGather -> combine ----
            stin1 = stp.tile([128, 3, 2], f32, tag="stin1")
            for c3 in range(3):
                nc.vector.reduce_sum(stin1[:, c3, 0:1], stats1[:, c3, :, 0:4], axis=AX.XY)
                nc.vector.reduce_sum(stin1[:, c3, 1:2], stats1[:, c3, :, 4:8], axis=AX.XY)
            ag1i = dram.tile([128, 3, 2], f32)
            ag1o = dram.tile([NCORES, 128, 6], f32)
            nc.sync.dma_start(ag1i, stin1)
            ag(ag1i.opt(), ag1o.opt(), ALL)
            g1 = stp.tile([128, 6, NCORES], f32, tag="g1")
            import concourse.bass as _b
            nc.sync.dma_start(
                g1, _b.AP(tensor=ag1o.tensor, offset=ag1o.offset,
                          ap=[[6, 128], [1, 6], [768, NCORES]]))
            tot1 = stp.tile([128, 6], f32, tag="tot1")
            nc.vector.reduce_sum(tot1, g1, axis=AX.X)
            mean1 = stp.tile([128, 3], f32, tag="mean1")
            nc.vector.tensor_scalar_mul(mean1, tot1[:, 0::2], 1.0 / CNT1)
            e2 = stp.tile([128, 3], f32, tag="e2")
            nc.vector.tensor_scalar_mul(e2, tot1[:, 1::2], 1.0 / CNT1)
            m2 = stp.tile([128, 3], f32, tag="m2")
            nc.vector.tensor_mul(m2, mean1, mean1)
            nc.vector.tensor_sub(e2, e2, m2)
            nc.scalar.activation(e2, e2, AF.Sqrt, bias=eps_sb)
            nc.vector.reciprocal(e2, e2)
            nc.vector.tensor_mul(scale1, gx_sb, e2)
            nc.vector.tensor_mul(m2, mean1, scale1)
            nc.vector.tensor_sub(shift1, btx_sb, m2)

            # ================= Phase 2: recurrence =================
            with tc.tile_pool(name="yp", bufs=4) as yp, \
                 tc.tile_pool(name="crawp", bufs=4) as crawp, \
                 tc.tile_pool(name="tp", bufs=2) as tp, \
                 tc.tile_pool(name="sq2", bufs=2) as sq2, \
                 tc.tile_pool(name="ps2", bufs=6, space="PSUM") as ps2, \
                 tc.tile_pool(name="pse2", bufs=2, space="PSUM") as pse2:

                # ---- t = 0: h0 = sigmoid(x_z) * tanh(x_h) ----
                y0z = yp.tile([128, ROWS, 64], f32, tag="yt")
                nc.sync.dma_start(y0z[:, 0:17, :], y_dram[0, 0, :, 0:17, :])
                nc.sync.dma_start(y0z[:, 17:34, :], y_dram[0, 0, :, 17:34, :])
                y0h = yp.tile([128, ROWS, 64], f32, tag="yt")
                nc.sync.dma_start(y0h[:, 0:17, :], y_dram[0, 2, :, 0:17, :])
                nc.sync.dma_start(y0h[:, 17:34, :], y_dram[0, 2, :, 17:34, :])
                z0 = tp.tile([128, OWN, 64], f32, tag="z")
                nc.scalar.activation(z0, y0z[:, 1:33, :], AF.Sigmoid,
                                     scale=scale1[:, 0:1], bias=shift1[:, 0:1])
                th0 = tp.tile([128, OWN, 64], f32, tag="d")
                nc.scalar.activation(th0, y0h[:, 1:33, :], AF.Tanh,
                                     scale=scale1[:, 2:3], bias=shift1[:, 2:3])
                nc.vector.tensor_mul(h_pad[:, 2:34, 1:65], z0, th0)
                nc.vector.tensor_copy(h_bf[:, 2:34, 1:65], h_pad[:, 2:34, 1:65])
                nc.sync.dma_start(outd[0], h_pad[:, 2:34, 1:65])

                for t in range(1, T):
                    # ---- halo AllGather of h edge rows between H-half pairs ----
                    hbi = dram2.tile([128, 4, 64], f32, tag="hbi")
                    nc.sync.dma_start(hbi[:, 0:2, :], h_pad[:, 2:4, 1:65])
                    nc.sync.dma_start(hbi[:, 2:4, :], h_pad[:, 32:34, 1:65])
                    hbo = dram2.tile([2, 128, 4, 64], f32, tag="hbo")
                    ag(hbi.opt(), hbo.opt(), PAIRS)
                    nc.sync.dma_start(h_pad[:, 0:2, 1:65], hbo[0, :, 2:4, :])
                    nc.vector.tensor_scalar_mul(h_pad[:, 0:2, 1:65],
                                                h_pad[:, 0:2, 1:65], mpre_sb)
                    nc.vector.tensor_copy(h_bf[:, 0:2, 1:65], h_pad[:, 0:2, 1:65])
                    nc.sync.dma_start(h_pad[:, 34:36, 1:65], hbo[1, :, 0:2, :])
                    nc.vector.tensor_scalar_mul(h_pad[:, 34:36, 1:65],
                                                h_pad[:, 34:36, 1:65], mpost_sb)
                    nc.vector.tensor_copy(h_bf[:, 34:36, 1:65], h_pad[:, 34:36, 1:65])

                    # ---- y loads ----
                    yz = yp.tile([128, ROWS, 64], f32, tag="yt")
                    nc.sync.dma_start(yz[:, 0:17, :], y_dram[t, 0, :, 0:17, :])
                    nc.sync.dma_start(yz[:, 17:34, :], y_dram[t, 0, :, 17:34, :])
                    yr = yp.tile([128, ROWS, 64], f32, tag="yt")
                    nc.sync.dma_start(yr[:, 0:17, :], y_dram[t, 1, :, 0:17, :])
                    nc.sync.dma_start(yr[:, 17:34, :], y_dram[t, 1, :, 17:34, :])
                    yh = yp.tile([128, ROWS, 64], f32, tag="yt")
                    nc.sync.dma_start(yh[:, 0:17, :], y_dram[t, 2, :, 0:17, :])
                    nc.sync.dma_start(yh[:, 17:34, :], y_dram[t, 2, :, 17:34, :])

                    # ---- conv_zr: 3x3, h -> 256ch, rows rel 1..34 ----
                    st_zr = stp.tile([128, 2, 8], f32, tag="stzr")
                    craw = []
                    for c2 in range(2):
                        cr = crawp.tile([128, ROWS, 64], f32, tag="craw")
                        craw.append(cr)
                        for ft in range(4):
                            i0 = 1 + 8 * ft
                            ps = ps2.tile([128, 8, 64], f32, tag="ps2")
                            k = 0
                            for ky in range(3):
                                for kx in range(3):
                                    nc.tensor.matmul(
                                        ps, wzr_sb[:, c2 * 9 + ky * 3 + kx, :],
                                        h_bf[:, i0 + ky:i0 + ky + 8, kx:kx + 64],
                                        start=(k == 0), stop=(k == 8))
                                    k += 1
                            nc.vector.tensor_scalar(
                                out=cr[:, i0:i0 + 8, :], in0=ps,
                                scalar1=1.0, scalar2=None, op0=ALU.mult, op1=ALU.add,
                                accum_out=st_zr[:, c2, ft:ft + 1])
                            sq = sq2.tile([128, 8, 64], f32, tag="sq2")
                            nc.scalar.activation(
                                sq, ps, AF.Square,
                                accum_out=st_zr[:, c2, 4 + ft:5 + ft])
                        pse = pse2.tile([128, 2, 64], f32, tag="pse2")
                        k = 0
                        for ky in range(3):
                            for kx in range(3):
                                nc.tensor.matmul(
                                    pse, wzr_sb[:, c2 * 9 + ky * 3 + kx, :],
                                    h_bf[:, ky::33, kx:kx + 64],
                                    start=(k == 0), stop=(k == 8))
                                k += 1
                        nc.vector.tensor_copy(craw[c2][:, 0:1, :], pse[:, 0:1, :])
                        nc.vector.tensor_copy(craw[c2][:, 33:34, :], pse[:, 1:2, :])

                    # ---- zr BN stats AllGather ----
                    stv = stp.tile([128, 2, 2], f32, tag="stv")
                    for c2 in range(2):
                        nc.vector.reduce_sum(stv[:, c2, 0:1], st_zr[:, c2, 0:4], axis=AX.X)
                        nc.vector.reduce_sum(stv[:, c2, 1:2], st_zr[:, c2, 4:8], axis=AX.X)
                    sti = dram2.tile([128, 2, 2], f32, tag="sti")
                    nc.sync.dma_start(sti, stv)
                    sto = dram2.tile([NCORES, 128, 4], f32, tag="sto")
                    ag(sti.opt(), sto.opt(), ALL)
                    g2 = stp.tile([128, 4, NCORES], f32, tag="g2")
                    nc.sync.dma_start(
                        g2, _b.AP(tensor=sto.tensor, offset=sto.offset,
                                  ap=[[4, 128], [1, 4], [512, NCORES]]))
                    tot2 = stp.tile([128, 4], f32, tag="tot2")
                    nc.vector.reduce_sum(tot2, g2, axis=AX.X)
                    mean2 = stp.tile([128, 2], f32, tag="mean2")
                    nc.vector.tensor_scalar_mul(mean2, tot2[:, 0::2], 1.0 / CNT2)
                    e22 = stp.tile([128, 2], f32, tag="e22")
                    nc.vector.tensor_scalar_mul(e22, tot2[:, 1::2], 1.0 / CNT2)
                    m22 = stp.tile([128, 2], f32, tag="m22")
                    nc.vector.tensor_mul(m22, mean2, mean2)
                    nc.vector.tensor_sub(e22, e22, m22)
                    nc.scalar.activation(e22, e22, AF.Sqrt, bias=eps_sb)
                    nc.vector.reciprocal(e22, e22)
                    sc2 = stp.tile([128, 2], f32, tag="sc2")
                    nc.vector.tensor_mul(sc2, gzr_sb, e22)
                    sh2 = stp.tile([128, 2], f32, tag="sh2")
                    nc.vector.tensor_mul(m22, mean2, sc2)
                    nc.vector.tensor_sub(sh2, btzr_sb, m22)

                    # ---- tmp = relu6(BN(conv)) + BN_x2h(y); z, r; rh ----
                    for c2 in range(2):
                        cr = craw[c2]
                        nc.scalar.activation(cr, cr, AF.Relu,
                                             scale=sc2[:, c2:c2 + 1],
                                             bias=sh2[:, c2:c2 + 1])
                        nc.vector.tensor_scalar_min(cr, cr, 6.0)
                        yt_ = yz if c2 == 0 else yr
                        nc.vector.tensor_scalar(
                            out=yt_, in0=yt_, scalar1=scale1[:, c2:c2 + 1],
                            scalar2=shift1[:, c2:c2 + 1],
                            op0=ALU.mult, op1=ALU.add)
                        nc.vector.tensor_add(cr, cr, yt_)
                    z = tp.tile([128, OWN, 64], f32, tag="z")
                    nc.scalar.activation(z, craw[0][:, 1:33, :], AF.Sigmoid)
                    r = craw[1]
                    nc.scalar.activation(r, r, AF.Sigmoid)
                    nc.vector.tensor_mul(rh_pad[:, 2:34, 1:65], r[:, 1:33, :],
                                         h_pad[:, 2:34, 1:65])
                    nc.vector.tensor_mul(rh_pad[:, 1:2, 1:65], r[:, 0:1, :],
                                         h_pad[:, 1:2, 1:65])
                    nc.vector.tensor_scalar_mul(rh_pad[:, 1:2, 1:65],
                                                rh_pad[:, 1:2, 1:65], mpre_sb)
                    nc.vector.tensor_mul(rh_pad[:, 34:35, 1:65], r[:, 33:34, :],
                                         h_pad[:, 34:35, 1:65])
                    nc.vector.tensor_scalar_mul(rh_pad[:, 34:35, 1:65],
                                                rh_pad[:, 34:35, 1:65], mpost_sb)

                    # ---- conv_hh: 3x3, rh -> 128ch, own rows ----
                    st_hh = stp.tile([128, 8], f32, tag="sthh")
                    ch = crawp.tile([128, ROWS, 64], f32, tag="craw")
                    for ft in range(4):
                        j0 = 8 * ft
                        ps = ps2.tile([128, 8, 64], f32, tag="ps2")
                        k = 0
                        for ky in range(3):
                            for kx in range(3):
                                nc.tensor.matmul(
                                    ps, whh_sb[:, ky * 3 + kx, :],
                                    rh_pad[:, j0 + 1 + ky:j0 + 1 + ky + 8, kx:kx + 64],
                                    start=(k == 0), stop=(k == 8))
                                k += 1
                        nc.vector.tensor_scalar(
                            out=ch[:, j0:j0 + 8, :], in0=ps,
                            scalar1=1.0, scalar2=None, op0=ALU.mult, op1=ALU.add,
                            accum_out=st_hh[:, ft:ft + 1])
                        sq = sq2.tile([128, 8, 64], f32, tag="sq2")
                        nc.scalar.activation(sq, ps, AF.Square,
                                             accum_out=st_hh[:, 4 + ft:5 + ft])

                    # ---- hh BN stats AllGather ----
                    stv2 = stp.tile([128, 2], f32, tag="stv2")
                    nc.vector.reduce_sum(stv2[:, 0:1], st_hh[:, 0:4], axis=AX.X)
                    nc.vector.reduce_sum(stv2[:, 1:2], st_hh[:, 4:8], axis=AX.X)
                    sti2 = dram2.tile([128, 2], f32, tag="sti2")
                    nc.sync.dma_start(sti2, stv2)
                    sto2 = dram2.tile([NCORES, 128, 2], f32, tag="sto2")
                    ag(sti2.opt(), sto2.opt(), ALL)
                    g3 = stp.tile([128, 2, NCORES], f32, tag="g3")
                    nc.sync.dma_start(
                        g3, _b.AP(tensor=sto2.tensor, offset=sto2.offset,
                                  ap=[[2, 128], [1, 2], [256, NCORES]]))
                    tot3 = stp.tile([128, 2], f32, tag="tot3")
                    nc.vector.reduce_sum(tot3, g3, axis=AX.X)
                    mean3 = stp.tile([128, 1], f32, tag="mean3")
                    nc.vector.tensor_scalar_mul(mean3, tot3[:, 0:1], 1.0 / CNT2)
                    e23 = stp.tile([128, 1], f32, tag="e23")
                    nc.vector.tensor_scalar_mul(e23, tot3[:, 1:2], 1.0 / CNT2)
                    m23 = stp.tile([128, 1], f32, tag="m23")
                    nc.vector.tensor_mul(m23, mean3, mean3)
                    nc.vector.tensor_sub(e23, e23, m23)
                    nc.scalar.activation(e23, e23, AF.Sqrt, bias=eps_sb)
                    nc.vector.reciprocal(e23, e23)
                    sc3 = stp.tile([128, 1], f32, tag="sc3")
                    nc.vector.tensor_mul(sc3, ghh_sb, e23)
                    sh3 = stp.tile([128, 1], f32, tag="sh3")
                    nc.vector.tensor_mul(m23, mean3, sc3)
                    nc.vector.tensor_sub(sh3, bthh_sb, m23)

                    # ---- hh -> tanh -> nh; update h in place ----
                    nc.scalar.activation(ch[:, 0:32, :], ch[:, 0:32, :], AF.Relu,
                                         scale=sc3, bias=sh3)
                    nc.vector.tensor_scalar_min(ch[:, 0:32, :], ch[:, 0:32, :], 6.0)
                    nc.vector.tensor_scalar(
                        out=yh, in0=yh, scalar1=scale1[:, 2:3],
                        scalar2=shift1[:, 2:3], op0=ALU.mult, op1=ALU.add)
                    nc.vector.tensor_add(ch[:, 0:32, :], ch[:, 0:32, :], yh[:, 1:33, :])
                    nc.scalar.activation(ch[:, 0:32, :], ch[:, 0:32, :], AF.Tanh)
                    d = tp.tile([128, OWN, 64], f32, tag="d")
                    nc.vector.tensor_sub(d, ch[:, 0:32, :], h_pad[:, 2:34, 1:65])
                    nc.vector.tensor_mul(d, z, d)
                    nc.vector.tensor_add(h_pad[:, 2:34, 1:65],
                                         h_pad[:, 2:34, 1:65], d)
                    nc.vector.tensor_copy(h_bf[:, 2:34, 1:65], h_pad[:, 2:34, 1:65])
                    nc.sync.dma_start(outd[t], h_pad[:, 2:34, 1:65])

    nc.compile()
    return nc


def get_prog():
    global _PROG
    if _PROG is None:
        _PROG = _build_program()
    return _PROG


def prep_in_maps(x, w_x2h, g_x2h, bt_x2h, w_zr, g_zr, bt_zr, w_hh, g_hh, bt_hh):
    """Shard + pre-transform inputs on the host. Returns list of per-core dicts."""
    x = np.asarray(x, np.float32)
    w_x2h = np.asarray(w_x2h, np.float32)
    w_zr = np.asarray(w_zr, np.float32)
    w_hh = np.asarray(w_hh, np.float32)

    # weights: pairs / singles for the 5x5, per-tap for the 3x3s
    wp = np.zeros((128, 30, 128), np.float32)
    ws = np.zeros((64, 15, 128), np.float32)
    for c3 in range(3):
        cs = slice(128 * c3, 128 * (c3 + 1))
        for ky in range(5):
            for e in range(2):
                kx = 2 * e
                wp[0:64, c3 * 10 + ky * 2 + e] = w_x2h[cs, :, ky, kx].T
                wp[64:128, c3 * 10 + ky * 2 + e] = w_x2h[cs, :, ky, kx + 1].T
            ws[:, c3 * 5 + ky] = w_x2h[cs, :, ky, 4].T
    wzr = np.zeros((128, 18, 128), np.float32)
    for c2 in range(2):
        cs = slice(128 * c2, 128 * (c2 + 1))
        for ky in range(3):
            for kx in range(3):
                wzr[:, c2 * 9 + ky * 3 + kx] = w_zr[cs, :, ky, kx].T
    whh = np.zeros((128, 9, 128), np.float32)
    for ky in range(3):
        for kx in range(3):
            whh[:, ky * 3 + kx] = w_hh[:, :, ky, kx].T

    gx = np.ascontiguousarray(np.asarray(g_x2h, np.float32).reshape(3, 128).T)
    btx = np.ascontiguousarray(np.asarray(bt_x2h, np.float32).reshape(3, 128).T)
    gzr = np.ascontiguousarray(np.asarray(g_zr, np.float32).reshape(2, 128).T)
    btzr = np.ascontiguousarray(np.asarray(bt_zr, np.float32).reshape(2, 128).T)
    ghh = np.asarray(g_hh, np.float32).reshape(128, 1)
    bthh = np.asarray(bt_hh, np.float32).reshape(128, 1)

    from ml_dtypes import bfloat16
    shared = dict(wp=wp.astype(bfloat16), ws=ws.astype(bfloat16),
                  wzr=wzr.astype(bfloat16), whh=whh.astype(bfloat16),
                  gx=gx, btx=btx, gzr=gzr, btzr=btzr, ghh=ghh, bthh=bthh)

    in_maps = []
    for c in range(NCORES):
        n, half = c // 2, c % 2
        base = 32 * half
        # padded input band: rows base-3..base+34, cols -2..65
        xpad = np.zeros((T, CIN, HP1, WP1), np.float32)
        i0 = max(0, base - 3)
        i1 = min(H, base + 35)
        j0 = i0 - (base - 3)
        xpad[:, :, j0:j0 + (i1 - i0), 2:66] = x[:, n, :, i0:i1, :]
        x2 = np.zeros((T, 128, HP1, WP1), np.float32)
        x2[:, 0:64] = xpad
        x2[:, 64:128, :, 0:WP1 - 1] = xpad[:, :, :, 1:WP1]
        m = dict(shared)
        m["x2"] = x2.astype(bfloat16)
        m["mpre"] = np.full((128, 1), 0.0 if half == 0 else 1.0, np.float32)
        m["mpost"] = np.full((128, 1), 1.0 if half == 0 else 0.0, np.float32)
        in_maps.append(m)
    return in_maps


def assemble_output(results):
    out = np.zeros((T, N, CD, H, W), np.float32)
    for c in range(NCORES):
        n, half = c // 2, c % 2
        out[:, n, :, 32 * half:32 * half + 32, :] = results[c]["out"]
    return out


def kernel(**inputs):
    from concourse import bass_utils
    nc = get_prog()
    in_maps = prep_in_maps(
        inputs["x"], inputs["w_x2h"], inputs["g_x2h"], inputs["bt_x2h"],
        inputs["w_zr"], inputs["g_zr"], inputs["bt_zr"],
        inputs["w_hh"], inputs["g_hh"], inputs["bt_hh"])
    res = bass_utils.run_bass_kernel_spmd(nc, in_maps, core_ids=list(range(NCORES)))
    return assemble_output(res.results)



# revision 19
# speedup vs baseline: 2.7191x; 1.4315x over previous
"""ConvGRU (nn_ConvRNN) Trainium2 Bass kernel — 8-core SPMD.

Sharding: 8 cores = (batch n in 0..3) x (H half in {top, bottom}). Each core
owns a 32-row band of one image for the whole pipeline:
  Phase 1: 5x5 conv 64->384 for all T=8 timesteps on the local band, raw conv
           output spilled to DRAM, per-channel sum/sumsq partials reduced
           on-chip, one 8-core AllGather for the global BN statistics.
  Phase 2: the GRU-style recurrence. Per step: a 2-row halo AllGather between
           H-half pairs, 3x3 convs (zr: 256ch, hh: 128ch) with one extra
           conv output row computed on each side (so the r*h halo row is
           produced locally), per-conv BN-stat AllGather.

Performance structure:
  - fp16 matmul operands (1 PE cycle/row vs 4 for fp32; fp32 PSUM accum).
  - Weight-stationary conv loops: taps outermost, each tap's weights loaded
    once and used by 5 back-to-back N=512 matmuls into different PSUM banks.
  - Phase-1 taps packed 25 -> 13 matmuls/tile: column pairs via a col-shifted
    channel copy, the kx=4 column paired across rows via a row-shifted copy.
  - No ACT table switches: Relu/Sigmoid/Tanh/Copy all live in one table set;
    sum-of-squares runs on GpSimd, rsqrt for BN runs on DVE (Newton).
  - Per-step critical path: r computed before z (z overlaps conv_hh), h edge
    rows updated first so the halo AllGather overlaps the interior update.

Conv biases are dropped: BN subtracts the batch mean, so a per-channel bias
added before BN cancels exactly.
"""

import numpy as np

T, N, CIN, H, W = 8, 4, 64, 64, 64
CD = 128
NCORES = 8
EPS = 1e-5

HP1, WP1 = 38, 68        # phase-1 padded input rows/cols per core
ROWS = 34                # conv output rows stored per core (rel 0..33)
OWN = 32                 # own rows
HPAD, WPAD = 36, 66      # phase-2 padded h tile
CNT1 = float(T * N * H * W)      # x2h BN count
CNT2 = float(N * H * W)          # recurrence BN count
MAGIC = 0x5F3759DF

_PROG = None


def _build_program():
    import concourse.bacc as bacc
    import concourse.bass as bass
    import concourse.tile as tile
    from concourse import mybir

    f32 = mybir.dt.float32
    i32 = mybir.dt.int32
    fp16 = mybir.dt.float16
    AF = mybir.ActivationFunctionType
    ALU = mybir.AluOpType
    AX = mybir.AxisListType
    PAIRS = [[2 * i, 2 * i + 1] for i in range(NCORES // 2)]
    ALL = [list(range(NCORES))]

    nc = bacc.Bacc("TRN2", target_bir_lowering=False, debug=False,
                   enable_asserts=False, num_devices=NCORES)

    x2d = nc.dram_tensor("x2", [T, 128, HP1, WP1], fp16, kind="ExternalInput")
    x3d = nc.dram_tensor("x3", [T, 128, HP1, WP1], fp16, kind="ExternalInput")
    wpd = nc.dram_tensor("wp", [128, 30, 128], fp16, kind="ExternalInput")
    wrpd = nc.dram_tensor("wrp", [128, 6, 128], fp16, kind="ExternalInput")
    wsd = nc.dram_tensor("ws", [64, 3, 128], fp16, kind="ExternalInput")
    wzrd = nc.dram_tensor("wzr", [128, 18, 128], fp16, kind="ExternalInput")
    whhd = nc.dram_tensor("whh", [128, 9, 128], fp16, kind="ExternalInput")
    gxd = nc.dram_tensor("gx", [128, 3], f32, kind="ExternalInput")
    btxd = nc.dram_tensor("btx", [128, 3], f32, kind="ExternalInput")
    gzrd = nc.dram_tensor("gzr", [128, 2], f32, kind="ExternalInput")
    btzrd = nc.dram_tensor("btzr", [128, 2], f32, kind="ExternalInput")
    ghhd = nc.dram_tensor("ghh", [128, 1], f32, kind="ExternalInput")
    bthhd = nc.dram_tensor("bthh", [128, 1], f32, kind="ExternalInput")
    mpred = nc.dram_tensor("mpre", [128, 1], f32, kind="ExternalInput")
    mpostd = nc.dram_tensor("mpost", [128, 1], f32, kind="ExternalInput")
    outd = nc.dram_tensor("out", [T, 128, OWN, 64], f32, kind="ExternalOutput")

    def ag(ins_ap, outs_ap, groups):
        nc.gpsimd.collective_compute(
            "AllGather", ALU.bypass, replica_groups=groups,
            ins=[ins_ap], outs=[outs_ap])

    with tile.TileContext(nc) as tc:
        with tc.tile_pool(name="consts", bufs=1) as consts, \
             tc.tile_pool(name="dram", bufs=1, space="DRAM") as dram, \
             tc.tile_pool(name="dram2", bufs=2, space="DRAM") as dram2, \
             tc.tile_pool(name="stp", bufs=2) as stp:

            # ---- persistent weights / consts ----
            wp_sb = consts.tile([128, 30, 128], fp16)
            nc.sync.dma_start(wp_sb, wpd[:])
            wrp_sb = consts.tile([128, 6, 128], fp16)
            nc.sync.dma_start(wrp_sb, wrpd[:])
            ws_sb = consts.tile([64, 3, 128], fp16)
            nc.sync.dma_start(ws_sb, wsd[:])
            wzr_sb = consts.tile([128, 18, 128], fp16)
            nc.sync.dma_start(wzr_sb, wzrd[:])
            whh_sb = consts.tile([128, 9, 128], fp16)
            nc.sync.dma_start(whh_sb, whhd[:])
            gx_sb = consts.tile([128, 3], f32)
            nc.sync.dma_start(gx_sb, gxd[:])
            btx_sb = consts.tile([128, 3], f32)
            nc.sync.dma_start(btx_sb, btxd[:])
            gzr_sb = consts.tile([128, 2], f32)
            nc.sync.dma_start(gzr_sb, gzrd[:])
            btzr_sb = consts.tile([128, 2], f32)
            nc.sync.dma_start(btzr_sb, btzrd[:])
            ghh_sb = consts.tile([128, 1], f32)
            nc.sync.dma_start(ghh_sb, ghhd[:])
            bthh_sb = consts.tile([128, 1], f32)
            nc.sync.dma_start(bthh_sb, bthhd[:])
            mpre_sb = consts.tile([128, 1], f32)
            nc.sync.dma_start(mpre_sb, mpred[:])
            mpost_sb = consts.tile([128, 1], f32)
            nc.sync.dma_start(mpost_sb, mpostd[:])

            magic_sb = consts.tile([128, 4], i32)
            nc.vector.memset(magic_sb, MAGIC)
            h_pad = consts.tile([128, HPAD, WPAD], f32)
            nc.vector.memset(h_pad, 0.0)
            rh_pad = consts.tile([128, HPAD, WPAD], fp16)
            nc.vector.memset(rh_pad, 0.0)
            h_bf = consts.tile([128, HPAD, WPAD], fp16)
            nc.vector.memset(h_bf, 0.0)
            junk = consts.tile([128, 8, 64], f32)

            y_dram = dram.tile([T, 3, 128, ROWS, 64], f32)

            stats1 = consts.tile([128, 3, T, 8], f32)
            scale1 = consts.tile([128, 3], f32)
            shift1 = consts.tile([128, 3], f32)

            def rsqrt_newton(out, v, k, tag):
                """out = 1/sqrt(v), v: [128,k] f32 (v>0). DVE only."""
                sh_i = stp.tile([128, k], i32, tag=tag + "i")
                nc.vector.tensor_scalar(
                    out=sh_i, in0=v.bitcast(i32), scalar1=1, scalar2=None,
                    op0=ALU.logical_shift_right)
                nc.vector.tensor_tensor(
                    out=sh_i, in0=magic_sb[:, 0:k], in1=sh_i, op=ALU.subtract)
                s = out
                nc.vector.tensor_copy(s, sh_i.bitcast(f32))
                a = stp.tile([128, k], f32, tag=tag + "a")
                for _ in range(2):
                    nc.vector.tensor_mul(a, s, s)
                    nc.vector.tensor_mul(a, a, v)
                    nc.vector.tensor_scalar(
                        out=a, in0=a, scalar1=-0.5, scalar2=1.5,
                        op0=ALU.mult, op1=ALU.add)
                    nc.vector.tensor_mul(s, s, a)

            def bn_scale_shift(sc, shv, tot, k, gamma, beta, cnt, tag):
                """tot: [128, 2k] (sum, sumsq interleaved stride 2).
                sc = gamma*rsqrt(var+eps); shv = beta - mean*sc."""
                mean = stp.tile([128, k], f32, tag=tag + "m")
                nc.vector.tensor_scalar_mul(mean, tot[:, 0::2], 1.0 / cnt)
                v = stp.tile([128, k], f32, tag=tag + "v")
                nc.vector.tensor_scalar_mul(v, tot[:, 1::2], 1.0 / cnt)
                m2 = stp.tile([128, k], f32, tag=tag + "m2")
                nc.vector.tensor_mul(m2, mean, mean)
                nc.vector.tensor_sub(v, v, m2)
                nc.vector.tensor_scalar_add(v, v, EPS)
                rs = stp.tile([128, k], f32, tag=tag + "rs")
                rsqrt_newton(rs, v, k, tag)
                nc.vector.tensor_mul(sc, gamma, rs)
                nc.vector.tensor_mul(m2, mean, sc)
                nc.vector.tensor_sub(shv, beta, m2)

            # ================= Phase 1: 5x5 conv 64->384 =================
            # 13 taps per (t, c3): 10 col-pairs (x2), 2 row-pairs (x3, kx=4),
            # 1 single (ws, ky=4 kx=4). Tap-outer: one weight load feeds
            # 4 interior tiles + 1 edge tile.
            with tc.tile_pool(name="x1", bufs=2) as x1, \
                 tc.tile_pool(name="x1b", bufs=2) as x1b, \
                 tc.tile_pool(name="y1", bufs=3) as y1, \
                 tc.tile_pool(name="ps1", bufs=6, space="PSUM") as ps1, \
                 tc.tile_pool(name="pse1", bufs=2, space="PSUM") as pse1:
                for t in range(T):
                    x2t = x1.tile([128, HP1, WP1], fp16, tag="x2t")
                    for r0, r1 in ((0, 13), (13, 26), (26, 38)):
                        nc.sync.dma_start(x2t[:, r0:r1, :], x2d[t, :, r0:r1, :])
                    x3t = x1b.tile([128, HP1, WP1], fp16, tag="x3t")
                    for r0, r1 in ((0, 13), (13, 26), (26, 38)):
                        nc.sync.dma_start(x3t[:, r0:r1, :], x3d[t, :, r0:r1, :])
                    for c3 in range(3):
                        taps = []
                        for ky in range(5):
                            for e in range(2):
                                taps.append((wp_sb[:, c3 * 10 + ky * 2 + e, :],
                                             0, ky, 2 * e))
                        for kyp in (0, 2):
                            taps.append((wrp_sb[:, c3 * 2 + kyp // 2, :],
                                         1, kyp, 4))
                        taps.append((ws_sb[:, c3, :], 2, 4, 4))

                        ps = [ps1.tile([128, 8, 64], f32, tag="ps1",
                                       name=f"ps1_{ft}")
                              for ft in range(4)]
                        pse = pse1.tile([128, 2, 64], f32, tag="pse")
                        nt = len(taps)
                        for j, (w, kind, ky, kx) in enumerate(taps):
                            st, sp = (j == 0), (j == nt - 1)
                            for ft in range(4):
                                i0 = 1 + 8 * ft
                                if kind == 0:
                                    mv = x2t[:, i0 + ky:i0 + ky + 8, kx:kx + 64]
                                elif kind == 1:
                                    mv = x3t[:, i0 + ky:i0 + ky + 8, 4:68]
                                else:
                                    mv = x2t[0:64, i0 + ky:i0 + ky + 8, 4:68]
                                nc.tensor.matmul(ps[ft], w, mv, start=st, stop=sp)
                            if kind == 0:
                                mve = x2t[:, ky::33, kx:kx + 64]
                            elif kind == 1:
                                mve = x3t[:, ky::33, 4:68]
                            else:
                                mve = x2t[0:64, ky::33, 4:68]
                            nc.tensor.matmul(pse, w, mve, start=st, stop=sp)

                        y_sb = y1.tile([128, ROWS, 64], f32, tag="y1t")
                        for ft in range(4):
                            i0 = 1 + 8 * ft
                            nc.vector.tensor_scalar(
                                out=y_sb[:, i0:i0 + 8, :], in0=ps[ft],
                                scalar1=1.0, scalar2=None, op0=ALU.mult,
                                op1=ALU.add,
                                accum_out=stats1[:, c3, t, ft:ft + 1])
                            nc.scalar.activation(
                                junk, ps[ft], AF.Square,
                                accum_out=stats1[:, c3, t, 4 + ft:5 + ft])
                        nc.scalar.copy(y_sb[:, 0:1, :], pse[:, 0:1, :])
                        nc.scalar.copy(y_sb[:, 33:34, :], pse[:, 1:2, :])
                        nc.sync.dma_start(y_dram[t, c3, :, 0:17, :],
                                          y_sb[:, 0:17, :])
                        nc.sync.dma_start(y_dram[t, c3, :, 17:34, :],
                                          y_sb[:, 17:34, :])

            # ---- phase-1 BN stats: local reduce -> AllGather -> combine ----
            import concourse.bass as _b
            stin1 = stp.tile([128, 3, 2], f32, tag="stin1")
            for c3 in range(3):
                nc.vector.reduce_sum(stin1[:, c3, 0:1], stats1[:, c3, :, 0:4],
                                     axis=AX.XY)
                nc.vector.reduce_sum(stin1[:, c3, 1:2], stats1[:, c3, :, 4:8],
                                     axis=AX.XY)
            ag1i = dram.tile([128, 3, 2], f32)
            ag1o = dram.tile([NCORES, 128, 6], f32)
            nc.sync.dma_start(ag1i, stin1)
            ag(ag1i.opt(), ag1o.opt(), ALL)
            g1 = stp.tile([128, 6, NCORES], f32, tag="g1")
            nc.sync.dma_start(
                g1, _b.AP(tensor=ag1o.tensor, offset=ag1o.offset,
                          ap=[[6, 128], [1, 6], [768, NCORES]]))
            tot1 = stp.tile([128, 6], f32, tag="tot1")
            nc.vector.reduce_sum(tot1, g1, axis=AX.X)
            bn_scale_shift(scale1, shift1, tot1, 3, gx_sb, btx_sb, CNT1, "s1")

            # ================= Phase 2: recurrence =================
            with tc.tile_pool(name="yp", bufs=6) as yp, \
                 tc.tile_pool(name="crawp", bufs=4) as crawp, \
                 tc.tile_pool(name="chp", bufs=2) as chp, \
                 tc.tile_pool(name="tp", bufs=2) as tp, \
                 tc.tile_pool(name="ps2", bufs=6, space="PSUM") as ps2, \
                 tc.tile_pool(name="pse2", bufs=2, space="PSUM") as pse2:

                # ---- t = 0: h0 = sigmoid(x_z) * tanh(x_h) ----
                y0z = yp.tile([128, ROWS, 64], f32, tag="yt")
                nc.sync.dma_start(y0z[:, 0:17, :], y_dram[0, 0, :, 0:17, :])
                nc.sync.dma_start(y0z[:, 17:34, :], y_dram[0, 0, :, 17:34, :])
                y0h = yp.tile([128, ROWS, 64], f32, tag="yt")
                nc.sync.dma_start(y0h[:, 0:17, :], y_dram[0, 2, :, 0:17, :])
                nc.sync.dma_start(y0h[:, 17:34, :], y_dram[0, 2, :, 17:34, :])
                def fire_halo():
                    # pack h edge rows, AllGather within the H-half pair
                    hbi = dram2.tile([128, 4, 64], f32, tag="hbi")
                    nc.sync.dma_start(hbi[:, 0:2, :], h_pad[:, 2:4, 1:65])
                    nc.sync.dma_start(hbi[:, 2:4, :], h_pad[:, 32:34, 1:65])
                    hbo = dram2.tile([2, 128, 4, 64], f32, tag="hbo")
                    ag(hbi.opt(), hbo.opt(), PAIRS)
                    return hbo

                def recv_halo(hbo):
                    nc.sync.dma_start(h_pad[:, 0:2, 1:65], hbo[0, :, 2:4, :])
                    nc.vector.tensor_scalar_mul(h_pad[:, 0:2, 1:65],
                                                h_pad[:, 0:2, 1:65], mpre_sb)
                    nc.vector.tensor_copy(h_bf[:, 0:2, 1:65],
                                          h_pad[:, 0:2, 1:65])
                    nc.sync.dma_start(h_pad[:, 34:36, 1:65], hbo[1, :, 0:2, :])
                    nc.vector.tensor_scalar_mul(h_pad[:, 34:36, 1:65],
                                                h_pad[:, 34:36, 1:65], mpost_sb)
                    nc.vector.tensor_copy(h_bf[:, 34:36, 1:65],
                                          h_pad[:, 34:36, 1:65])

                z0 = tp.tile([128, OWN, 64], f32, tag="z")
                nc.scalar.activation(z0, y0z[:, 1:33, :], AF.Sigmoid,
                                     scale=scale1[:, 0:1], bias=shift1[:, 0:1])
                th0 = tp.tile([128, OWN, 64], f32, tag="d")
                nc.scalar.activation(th0, y0h[:, 1:33, :], AF.Tanh,
                                     scale=scale1[:, 2:3], bias=shift1[:, 2:3])
                nc.vector.tensor_mul(h_pad[:, 2:34, 1:65], z0, th0)
                hbo_cur = fire_halo()
                nc.vector.tensor_copy(h_bf[:, 2:34, 1:65], h_pad[:, 2:34, 1:65])
                nc.sync.dma_start(outd[0], h_pad[:, 2:34, 1:65])

                for t in range(1, T):
                    # ---- receive halo fired at the end of the previous step
                    recv_halo(hbo_cur)

                    # ---- y loads + BN-affine (GpSimd, off critical path) ----
                    yz = yp.tile([128, ROWS, 64], f32, tag="yt")
                    nc.sync.dma_start(yz[:, 0:17, :], y_dram[t, 0, :, 0:17, :])
                    nc.sync.dma_start(yz[:, 17:34, :], y_dram[t, 0, :, 17:34, :])
                    yr = yp.tile([128, ROWS, 64], f32, tag="yt")
                    nc.sync.dma_start(yr[:, 0:17, :], y_dram[t, 1, :, 0:17, :])
                    nc.sync.dma_start(yr[:, 17:34, :], y_dram[t, 1, :, 17:34, :])
                    yh = yp.tile([128, ROWS, 64], f32, tag="yt")
                    nc.sync.dma_start(yh[:, 0:17, :], y_dram[t, 2, :, 0:17, :])
                    nc.sync.dma_start(yh[:, 17:34, :], y_dram[t, 2, :, 17:34, :])
                    for yt_, c in ((yz, 0), (yr, 1), (yh, 2)):
                        nc.gpsimd.tensor_scalar(
                            out=yt_, in0=yt_, scalar1=scale1[:, c:c + 1],
                            scalar2=shift1[:, c:c + 1],
                            op0=ALU.mult, op1=ALU.add)

                    # ---- conv_zr: 3x3, h -> 256ch, tap-outer ----
                    st_zr = stp.tile([128, 2, 8], f32, tag="stzr")
                    craw = []
                    for c2 in range(2):
                        cr = crawp.tile([128, ROWS, 64], f32, tag="craw")
                        craw.append(cr)
                        ps = [ps2.tile([128, 8, 64], f32, tag="ps2",
                                       name=f"ps2_{ft}")
                              for ft in range(4)]
                        pse = pse2.tile([128, 2, 64], f32, tag="pse2")
                        for j in range(9):
                            ky, kx = j // 3, j % 3
                            w = wzr_sb[:, c2 * 9 + j, :]
                            st, sp = (j == 0), (j == 8)
                            for ft in range(4):
                                i0 = 1 + 8 * ft
                                nc.tensor.matmul(
                                    ps[ft], w,
                                    h_bf[:, i0 + ky:i0 + ky + 8, kx:kx + 64],
                                    start=st, stop=sp)
                            nc.tensor.matmul(pse, w, h_bf[:, ky::33, kx:kx + 64],
                                             start=st, stop=sp)
                        for ft in range(4):
                            i0 = 1 + 8 * ft
                            nc.vector.tensor_scalar(
                                out=cr[:, i0:i0 + 8, :], in0=ps[ft],
                                scalar1=1.0, scalar2=None, op0=ALU.mult,
                                op1=ALU.add,
                                accum_out=st_zr[:, c2, ft:ft + 1])
                            nc.scalar.activation(
                                junk, ps[ft], AF.Square,
                                accum_out=st_zr[:, c2, 4 + ft:5 + ft])
                        nc.scalar.copy(cr[:, 0:1, :], pse[:, 0:1, :])
                        nc.scalar.copy(cr[:, 33:34, :], pse[:, 1:2, :])

                    # ---- zr BN stats AllGather ----
                    stv = stp.tile([128, 2, 2], f32, tag="stv")
                    for c2 in range(2):
                        nc.vector.reduce_sum(stv[:, c2, 0:1],
                                             st_zr[:, c2, 0:4], axis=AX.X)
                        nc.vector.reduce_sum(stv[:, c2, 1:2],
                                             st_zr[:, c2, 4:8], axis=AX.X)
                    sti = dram2.tile([128, 2, 2], f32, tag="sti")
                    nc.sync.dma_start(sti, stv)
                    sto = dram2.tile([NCORES, 128, 4], f32, tag="sto")
                    ag(sti.opt(), sto.opt(), ALL)
                    g2 = stp.tile([128, 4, NCORES], f32, tag="g2")
                    nc.sync.dma_start(
                        g2, _b.AP(tensor=sto.tensor, offset=sto.offset,
                                  ap=[[4, 128], [1, 4], [512, NCORES]]))
                    tot2 = stp.tile([128, 4], f32, tag="tot2")
                    nc.vector.reduce_sum(tot2, g2, axis=AX.X)
                    sc2 = stp.tile([128, 2], f32, tag="sc2")
                    sh2 = stp.tile([128, 2], f32, tag="sh2")
                    bn_scale_shift(sc2, sh2, tot2, 2, gzr_sb, btzr_sb, CNT2,
                                   "s2")

                    # ---- r chain first (conv_hh needs it), z overlaps ----
                    tr = crawp.tile([128, ROWS, 64], f32, tag="craw")
                    nc.scalar.activation(tr, craw[1], AF.Relu,
                                         scale=sc2[:, 1:2], bias=sh2[:, 1:2])
                    tz = crawp.tile([128, ROWS, 64], f32, tag="craw")
                    nc.scalar.activation(tz, craw[0], AF.Relu,
                                         scale=sc2[:, 0:1], bias=sh2[:, 0:1])
                    # tmp = min(relu(...), 6) + y'
                    nc.vector.scalar_tensor_tensor(
                        out=tr, in0=tr, scalar=6.0, in1=yr,
                        op0=ALU.min, op1=ALU.add)
                    r = tr
                    nc.scalar.activation(r, tr, AF.Sigmoid)
                    nc.vector.tensor_mul(rh_pad[:, 2:34, 1:65], r[:, 1:33, :],
                                         h_pad[:, 2:34, 1:65])
                    nc.vector.tensor_mul(rh_pad[:, 1:2, 1:65], r[:, 0:1, :],
                                         h_pad[:, 1:2, 1:65])
                    nc.vector.tensor_mul(rh_pad[:, 34:35, 1:65], r[:, 33:34, :],
                                         h_pad[:, 34:35, 1:65])
                    # z chain (overlaps conv_hh)
                    nc.vector.scalar_tensor_tensor(
                        out=tz, in0=tz, scalar=6.0, in1=yz,
                        op0=ALU.min, op1=ALU.add)
                    z = tp.tile([128, OWN, 64], f32, tag="z")
                    nc.scalar.activation(z, tz[:, 1:33, :], AF.Sigmoid)

                    # ---- conv_hh: 3x3, rh -> 128ch, tap-outer ----
                    st_hh = stp.tile([128, 8], f32, tag="sthh")
                    ch = chp.tile([128, OWN, 64], f32, tag="crawh")
                    psh = [ps2.tile([128, 8, 64], f32, tag="ps2",
                                    name=f"psh_{ft}")
                           for ft in range(4)]
                    for j in range(9):
                        ky, kx = j // 3, j % 3
                        w = whh_sb[:, j, :]
                        st, sp = (j == 0), (j == 8)
                        for ft in range(4):
                            j0 = 8 * ft
                            nc.tensor.matmul(
                                psh[ft], w,
                                rh_pad[:, j0 + 1 + ky:j0 + 1 + ky + 8,
                                       kx:kx + 64],
                                start=st, stop=sp)
                    for ft in range(4):
                        j0 = 8 * ft
                        nc.vector.tensor_scalar(
                            out=ch[:, j0:j0 + 8, :], in0=psh[ft],
                            scalar1=1.0, scalar2=None, op0=ALU.mult,
                            op1=ALU.add, accum_out=st_hh[:, ft:ft + 1])
                        nc.scalar.activation(
                            junk, psh[ft], AF.Square,
                            accum_out=st_hh[:, 4 + ft:5 + ft])

                    # ---- hh BN stats AllGather ----
                    stv2 = stp.tile([128, 2], f32, tag="stv2")
                    nc.vector.reduce_sum(stv2[:, 0:1], st_hh[:, 0:4], axis=AX.X)
                    nc.vector.reduce_sum(stv2[:, 1:2], st_hh[:, 4:8], axis=AX.X)
                    sti2 = dram2.tile([128, 2], f32, tag="sti2")
                    nc.sync.dma_start(sti2, stv2)
                    sto2 = dram2.tile([NCORES, 128, 2], f32, tag="sto2")
                    ag(sti2.opt(), sto2.opt(), ALL)
                    g3 = stp.tile([128, 2, NCORES], f32, tag="g3")
                    nc.sync.dma_start(
                        g3, _b.AP(tensor=sto2.tensor, offset=sto2.offset,
                                  ap=[[2, 128], [1, 2], [256, NCORES]]))
                    tot3 = stp.tile([128, 2], f32, tag="tot3")
                    nc.vector.reduce_sum(tot3, g3, axis=AX.X)
                    sc3 = stp.tile([128, 1], f32, tag="sc3")
                    sh3 = stp.tile([128, 1], f32, tag="sh3")
                    bn_scale_shift(sc3, sh3, tot3, 1, ghh_sb, bthh_sb, CNT2,
                                   "s3")

                    # ---- hh -> tanh -> nh; edge rows first, then halo ----
                    nc.scalar.activation(ch, ch, AF.Relu, scale=sc3, bias=sh3)
                    nc.vector.scalar_tensor_tensor(
                        out=ch, in0=ch, scalar=6.0, in1=yh[:, 1:33, :],
                        op0=ALU.min, op1=ALU.add)
                    nc.scalar.activation(ch, ch, AF.Tanh)
                    d = tp.tile([128, OWN, 64], f32, tag="d")
                    # edge chunks (own rows 0:2 and 30:32) first, so the halo
                    # AllGather overlaps the interior update
                    for r0, r1 in ((0, 2), (30, 32), (2, 30)):
                        hs = h_pad[:, 2 + r0:2 + r1, 1:65]
                        nc.vector.tensor_sub(d[:, r0:r1, :], ch[:, r0:r1, :],
                                             hs)
                        nc.vector.tensor_mul(d[:, r0:r1, :], z[:, r0:r1, :],
                                             d[:, r0:r1, :])
                        nc.vector.tensor_add(hs, hs, d[:, r0:r1, :])
                        if (r0, r1) == (30, 32) and t < T - 1:
                            hbo_cur = fire_halo()
                    nc.vector.tensor_copy(h_bf[:, 2:34, 1:65],
                                          h_pad[:, 2:34, 1:65])
                    nc.sync.dma_start(outd[t], h_pad[:, 2:34, 1:65])

    nc.compile()
    return nc


def get_prog():
    global _PROG
    if _PROG is None:
        _PROG = _build_program()
    return _PROG


def prep_in_maps(x, w_x2h, g_x2h, bt_x2h, w_zr, g_zr, bt_zr, w_hh, g_hh, bt_hh):
    """Shard + pre-transform inputs on the host. Returns list of per-core dicts."""
    x = np.asarray(x, np.float32)
    w_x2h = np.asarray(w_x2h, np.float32)
    w_zr = np.asarray(w_zr, np.float32)
    w_hh = np.asarray(w_hh, np.float32)

    # 5x5 weights: col-pairs (wp), row-pairs for kx=4 (wrp), single (ws)
    wp = np.zeros((128, 30, 128), np.float32)
    wrp = np.zeros((128, 6, 128), np.float32)
    ws = np.zeros((64, 3, 128), np.float32)
    for c3 in range(3):
        cs = slice(128 * c3, 128 * (c3 + 1))
        for ky in range(5):
            for e in range(2):
                kx = 2 * e
                wp[0:64, c3 * 10 + ky * 2 + e] = w_x2h[cs, :, ky, kx].T
                wp[64:128, c3 * 10 + ky * 2 + e] = w_x2h[cs, :, ky, kx + 1].T
        for kyp in (0, 2):
            wrp[0:64, c3 * 2 + kyp // 2] = w_x2h[cs, :, kyp, 4].T
            wrp[64:128, c3 * 2 + kyp // 2] = w_x2h[cs, :, kyp + 1, 4].T
        ws[:, c3] = w_x2h[cs, :, 4, 4].T
    wzr = np.zeros((128, 18, 128), np.float32)
    for c2 in range(2):
        cs = slice(128 * c2, 128 * (c2 + 1))
        for ky in range(3):
            for kx in range(3):
                wzr[:, c2 * 9 + ky * 3 + kx] = w_zr[cs, :, ky, kx].T
    whh = np.zeros((128, 9, 128), np.float32)
    for ky in range(3):
        for kx in range(3):
            whh[:, ky * 3 + kx] = w_hh[:, :, ky, kx].T

    gx = np.ascontiguousarray(np.asarray(g_x2h, np.float32).reshape(3, 128).T)
    btx = np.ascontiguousarray(np.asarray(bt_x2h, np.float32).reshape(3, 128).T)
    gzr = np.ascontiguousarray(np.asarray(g_zr, np.float32).reshape(2, 128).T)
    btzr = np.ascontiguousarray(np.asarray(bt_zr, np.float32).reshape(2, 128).T)
    ghh = np.asarray(g_hh, np.float32).reshape(128, 1)
    bthh = np.asarray(bt_hh, np.float32).reshape(128, 1)

    f16 = np.float16
    shared = dict(wp=wp.astype(f16), wrp=wrp.astype(f16), ws=ws.astype(f16),
                  wzr=wzr.astype(f16), whh=whh.astype(f16),
                  gx=gx, btx=btx, gzr=gzr, btzr=btzr, ghh=ghh, bthh=bthh)

    in_maps = []
    for c in range(NCORES):
        n, half = c // 2, c % 2
        base = 32 * half
        # padded input band: rows base-3..base+34, cols -2..65
        xpad = np.zeros((T, CIN, HP1, WP1), np.float32)
        i0 = max(0, base - 3)
        i1 = min(H, base + 35)
        j0 = i0 - (base - 3)
        xpad[:, :, j0:j0 + (i1 - i0), 2:66] = x[:, n, :, i0:i1, :]
        x2 = np.zeros((T, 128, HP1, WP1), np.float32)
        x2[:, 0:64] = xpad
        x2[:, 64:128, :, 0:WP1 - 1] = xpad[:, :, :, 1:WP1]
        x3 = np.zeros((T, 128, HP1, WP1), np.float32)
        x3[:, 0:64] = xpad
        x3[:, 64:128, 0:HP1 - 1, :] = xpad[:, :, 1:HP1, :]
        m = dict(shared)
        m["x2"] = x2.astype(f16)
        m["x3"] = x3.astype(f16)
        m["mpre"] = np.full((128, 1), 0.0 if half == 0 else 1.0, np.float32)
        m["mpost"] = np.full((128, 1), 1.0 if half == 0 else 0.0, np.float32)
        in_maps.append(m)
    return in_maps


def assemble_output(results):
    out = np.zeros((T, N, CD, H, W), np.float32)
    for c in range(NCORES):
        n, half = c // 2, c % 2
        out[:, n, :, 32 * half:32 * half + 32, :] = results[c]["out"]
    return out


def kernel(**inputs):
    from concourse import bass_utils
    nc = get_prog()
    in_maps = prep_in_maps(
        inputs["x"], inputs["w_x2h"], inputs["g_x2h"], inputs["bt_x2h"],
        inputs["w_zr"], inputs["g_zr"], inputs["bt_zr"],
        inputs["w_hh"], inputs["g_hh"], inputs["bt_hh"])
    res = bass_utils.run_bass_kernel_spmd(nc, in_maps, core_ids=list(range(NCORES)))
    return assemble_output(res.results)
